# revision 1
# baseline (speedup 1.0000x reference)
"""Causal self-attention (GQA + RMS-norm + RoPE) Trainium2 Bass kernel.

Sharding: 8 cores = 4 batches x 2 head-groups (tensor-parallel over heads).
Core c = 2*b + t handles batch b with Q heads [8t, 8t+8) and KV heads
[2t, 2t+2). Each core computes a partial output projection (its heads'
rows of W_out); the host sums the two partials per batch.

All matmuls run as float32r (full fp32 data, full-rate PE mode).

Pipeline per core:
  P1: qkv = x @ W_shard (transposed-x input), RMS+RoPE on q/k in natural
      layout, PE-transpose q/k to [d, tok], spill qT/kT/v to DRAM scratch.
  P2: per 512-token query window, per head: scoresT = kT_tile.T @ qT_win,
      +tri-mask on diagonal tiles, exp (ACT, scale=hd^-0.5), then
      yT += v_tile.T @ expT and sums += ones.T @ expT; normalize yT by
      broadcasted 1/sums.
  P3: out = sum_h yT_norm_h.T @ W_out_h rows -> partial [S, D].
"""
import sys, os
sys.path.insert(0, '/opt/trn_rl_repo')
import numpy as np

from concourse import bass, bacc, mybir, tile

f32 = mybir.dt.float32
f32r = mybir.dt.float32r

B, S, D = 4, 2048, 2048
H, HKV, HD = 16, 4, 128
HLOC = H // 2          # 8 q heads per core
KVLOC = HKV // 2       # 2 kv heads per core
SCALE = float(HD) ** -0.5
RMS_EPS = float(np.finfo(np.float32).eps)
ROPE_BASE = 10000.0

NTC = S // 128         # 16 token tiles
NDT = D // 128         # 16 contraction tiles
NWIN = S // 512        # 4 query windows


def _rope_tables():
    inv_freq = (1.0 / (ROPE_BASE ** (np.arange(0, HD, 2, dtype=np.float32) / HD))).astype(np.float32)
    freqs = np.arange(S, dtype=np.float32)[:, None] * inv_freq[None, :]
    cos = np.cos(freqs).astype(np.float32)
    sin = np.sin(freqs).astype(np.float32)
    cos2 = np.concatenate([cos, cos], axis=1)        # [S, 128]
    sin2 = np.concatenate([sin, -sin], axis=1)       # [S, 128]
    return cos2, sin2


def _tri_masks():
    # mask[vi][p, f] = -1e30 where kv > q for scoresT diag tiles:
    # kv = 128*j + p, q = 512*w + f, vi = j - 4*w -> masked iff p + 128*vi > f
    m = np.zeros((4, 128, 512), dtype=np.float32)
    p = np.arange(128)[:, None]
    f = np.arange(512)[None, :]
    for vi in range(4):
        m[vi][(p + 128 * vi) > f] = -1e30
    return m


def _emit_rms_rope(nc, scr, psum_ap, nheads, cos1, sin1, nat_tile, eps_ap):
    """psum_ap: [128, nheads*128] qkv psum slice; writes RMS+RoPE result into
    nat_tile (SBUF). cos1/sin1: [128, 1, 128] APs (cos duplicated, [sin,-sin]).

    rot(q) = q*cos2 + swap_halves(q)*sin2;  out = rot(q) * rsqrt(mean(q^2)+eps)
    rsqrt computed as exp(-0.5*ln(ss/128+eps)) on ACT (DVE reciprocal is slow).
    """
    w = nheads * 128
    sq = scr.tile([128, w], f32, tag="sq")
    nc.scalar.activation(sq[:], psum_ap, mybir.ActivationFunctionType.Square)
    ss = scr.tile([128, nheads, 1], f32, tag="ss")
    nc.vector.tensor_reduce(
        ss[:], sq[:].rearrange("p (h f) -> p h f", h=nheads),
        axis=mybir.AxisListType.X, op=mybir.AluOpType.add)
    lg = scr.tile([128, nheads, 1], f32, tag="lg")
    nc.scalar.activation(lg[:], ss[:], mybir.ActivationFunctionType.Ln,
                         scale=1.0 / HD, bias=eps_ap)
    rinv = scr.tile([128, nheads, 1, 1], f32, tag="rinv")
    nc.scalar.activation(rinv[:], lg[:], mybir.ActivationFunctionType.Exp,
                         scale=-0.5)

    shp = [128, nheads, 2, 64]
    p4 = psum_ap.rearrange("p (h x f) -> p h x f", h=nheads, x=2)
    p4s = p4[:, :, ::-1, :]
    cb = cos1.rearrange("p t (x f) -> p t x f", x=2).to_broadcast(shp)
    sb_ = sin1.rearrange("p t (x f) -> p t x f", x=2).to_broadcast(shp)
    rb = rinv[:].to_broadcast(shp)
    t1 = scr.tile(shp, f32, tag="t1")
    t2 = scr.tile(shp, f32, tag="t2")
    nc.vector.tensor_mul(t1[:], p4, cb)
    nc.vector.tensor_mul(t2[:], p4s, sb_)
    nc.vector.tensor_add(t1[:], t1[:], t2[:])
    nc.vector.tensor_mul(nat_tile[:].rearrange("p (h x f) -> p h x f", h=nheads, x=2),
                         t1[:], rb)


def build_program():
    cos_np, sin_np = _rope_tables()
    masks_np = _tri_masks()

    nc = bacc.Bacc(trn_type="TRN2")

    xt_d = nc.dram_tensor("xt", [D, S], f32, kind="ExternalInput")
    wq_d = nc.dram_tensor("wq", [D, HLOC * HD], f32, kind="ExternalInput")
    wkv_d = nc.dram_tensor("wkv", [D, 2 * KVLOC * HD], f32, kind="ExternalInput")
    wo_d = nc.dram_tensor("wo", [HLOC * HD, D], f32, kind="ExternalInput")
    out_d = nc.dram_tensor("out", [S, D], f32, kind="ExternalOutput")

    cos_d = nc.inline_tensor(cos_np, "cos_t")
    sin_d = nc.inline_tensor(sin_np, "sin_t")
    ident_d = nc.inline_tensor(np.eye(128, dtype=np.float32), "ident")
    masks_d = nc.inline_tensor(masks_np, "tri_masks")
    onescol_d = nc.inline_tensor(np.ones((128, 1), dtype=np.float32), "onescol")
    onesrow_d = nc.inline_tensor(np.ones((1, 128), dtype=np.float32), "onesrow")

    qt_scr = nc.dram_tensor("qt_scr", [HLOC, 128, S], f32)
    kt_scr = nc.dram_tensor("kt_scr", [KVLOC, 128, S], f32)
    v_scr = nc.dram_tensor("v_scr", [S, KVLOC * HD], f32)

    with tile.TileContext(nc) as tc:
        with tc.tile_pool(name="cst", bufs=1) as cst:
            cos_sb = cst.tile([128, NTC, 128], f32, tag="cos")
            sin_sb = cst.tile([128, NTC, 128], f32, tag="sin")
            ident = cst.tile([128, 128], f32, tag="ident")
            masks = cst.tile([128, 4, 512], f32, tag="masks")
            ones = cst.tile([128, 1], f32r, tag="ones")
            ones_r = cst.tile([1, 128], f32r, tag="ones_r")
            eps_sb = cst.tile([128, 1], f32, tag="eps")
            nc.sync.dma_start(out=cos_sb[:], in_=cos_d[:].rearrange("(t p) f -> p t f", p=128))
            nc.sync.dma_start(out=sin_sb[:], in_=sin_d[:].rearrange("(t p) f -> p t f", p=128))
            nc.sync.dma_start(out=ident[:], in_=ident_d[:])
            nc.sync.dma_start(out=masks[:], in_=masks_d[:].rearrange("v p f -> p v f"))
            nc.sync.dma_start(out=ones[:], in_=onescol_d[:].bitcast(f32r))
            nc.sync.dma_start(out=ones_r[:], in_=onesrow_d[:].bitcast(f32r))
            nc.gpsimd.memset(eps_sb[:], RMS_EPS)

            # ---------------- Phase 1: QKV projection ----------------
            with tc.tile_pool(name="w1", bufs=1) as w1, \
                 tc.tile_pool(name="xs", bufs=3) as xs, \
                 tc.tile_pool(name="nat", bufs=3) as nat, \
                 tc.tile_pool(name="stg", bufs=4) as stg, \
                 tc.tile_pool(name="p1a", bufs=6, space="PSUM") as p1a, \
                 tc.tile_pool(name="p1t", bufs=2, space="PSUM") as p1t:

                wq_sb = w1.tile([128, NDT, HLOC * HD], f32r, tag="wq")
                wkv_sb = w1.tile([128, NDT, 512], f32r, tag="wkv")
                wq_r = wq_d[:].bitcast(f32r).rearrange("(t p) c -> p t c", p=128)
                wkv_r = wkv_d[:].bitcast(f32r).rearrange("(t p) c -> p t c", p=128)
                for dt in range(NDT):
                    nc.sync.dma_start(out=wkv_sb[:, dt, :], in_=wkv_r[:, dt, :])
                for dt in range(NDT):
                    nc.scalar.dma_start(out=wq_sb[:, dt, :], in_=wq_r[:, dt, :])

                for tcid in range(NTC):
                    xt_sb = xs.tile([128, NDT, 128], f32r, tag="xt")
                    nc.sync.dma_start(
                        out=xt_sb[:],
                        in_=xt_d[:, tcid * 128:(tcid + 1) * 128]
                            .bitcast(f32r).rearrange("(t p) s -> p t s", p=128))

                    ps_q1 = p1a.tile([128, 512], f32, tag="acc")
                    ps_q2 = p1a.tile([128, 512], f32, tag="acc")
                    ps_kv = p1a.tile([128, 512], f32, tag="acc")
                    for dt in range(NDT):
                        st, sp = dt == 0, dt == NDT - 1
                        nc.tensor.matmul(ps_kv[:], xt_sb[:, dt, :], wkv_sb[:, dt, :], start=st, stop=sp)
                    for dt in range(NDT):
                        st, sp = dt == 0, dt == NDT - 1
                        lhs = xt_sb[:, dt, :]
                        nc.tensor.matmul(ps_q1[:], lhs, wq_sb[:, dt, 0:512], start=st, stop=sp)
                        nc.tensor.matmul(ps_q2[:], lhs, wq_sb[:, dt, 512:1024], start=st, stop=sp)

                    cos1 = cos_sb[:, tcid:tcid + 1, :]
                    sin1 = sin_sb[:, tcid:tcid + 1, :]

                    # q heads 0-3 / 4-7: RMS+RoPE, then PE-transpose to qT
                    for gi, ps in ((0, ps_q1), (1, ps_q2)):
                        qn = nat.tile([128, 512], f32, tag="qn")
                        _emit_rms_rope(nc, nat, ps[:], 4, cos1, sin1, qn, eps_sb[:])
                        for hh in range(4):
                            h = gi * 4 + hh
                            tp = p1t.tile([128, 128], f32, tag="tp")
                            nc.tensor.transpose(tp[:], qn[:, hh * 128:(hh + 1) * 128], ident[:])
                            sg = stg.tile([128, 128], f32, tag="sg")
                            nc.vector.tensor_copy(sg[:], tp[:])
                            nc.scalar.dma_start(
                                out=qt_scr[h][:, tcid * 128:(tcid + 1) * 128], in_=sg[:])

                    # k heads (cols 0:256 of kv psum)
                    kn = nat.tile([128, 256], f32, tag="kn")
                    _emit_rms_rope(nc, nat, ps_kv[:, 0:256], 2, cos1, sin1, kn, eps_sb[:])
                    for kh in range(KVLOC):
                        tp = p1t.tile([128, 128], f32, tag="tp")
                        nc.tensor.transpose(tp[:], kn[:, kh * 128:(kh + 1) * 128], ident[:])
                        sg = stg.tile([128, 128], f32, tag="sg")
                        nc.vector.tensor_copy(sg[:], tp[:])
                        nc.scalar.dma_start(
                            out=kt_scr[kh][:, tcid * 128:(tcid + 1) * 128], in_=sg[:])

                    # v: plain copy out (natural layout)
                    vn = nat.tile([128, 256], f32, tag="vn")
                    nc.vector.tensor_copy(vn[:], ps_kv[:, 256:512])
                    nc.scalar.dma_start(
                        out=v_scr[tcid * 128:(tcid + 1) * 128, :], in_=vn[:])

            # ------------- Phases 2+3 (share the ytn resident) -------------
            with tc.tile_pool(name="ytp", bufs=1) as ytp:
                ytn = ytp.tile([128, HLOC, S], f32r, tag="ytn")

                # ---- Phase 2: attention ----
                with tc.tile_pool(name="kv2", bufs=1) as kv2, \
                     tc.tile_pool(name="qw", bufs=2) as qw, \
                     tc.tile_pool(name="ex", bufs=6) as ex, \
                     tc.tile_pool(name="sm", bufs=4) as sm, \
                     tc.tile_pool(name="p2s", bufs=3, space="PSUM") as p2s, \
                     tc.tile_pool(name="p2y", bufs=3, space="PSUM") as p2y, \
                     tc.tile_pool(name="p2n", bufs=2, space="PSUM") as p2n:

                    kt_sb = kv2.tile([128, KVLOC, S], f32r, tag="kt")
                    v_sb = kv2.tile([128, NTC, KVLOC * HD], f32r, tag="v")
                    nc.sync.dma_start(out=kt_sb[:], in_=kt_scr[:].bitcast(f32r).rearrange("k p t -> p k t"))
                    nc.sync.dma_start(out=v_sb[:], in_=v_scr[:].bitcast(f32r).rearrange("(t p) c -> p t c", p=128))

                    for w in range(NWIN):
                        qt_win = qw.tile([128, HLOC, 512], f32r, tag="qtw")
                        nc.sync.dma_start(
                            out=qt_win[:],
                            in_=qt_scr[:, :, w * 512:(w + 1) * 512].bitcast(f32r).rearrange("h p t -> p h t"))
                        njt = 4 * w + 4
                        for hq in range(HLOC):
                            kvh = hq // 4
                            ps_y = p2y.tile([128, 512], f32, tag="y")
                            ps_s = p2n.tile([1, 512], f32, tag="s")
                            rhs_q = qt_win[:, hq, :]
                            for j in range(njt):
                                ps_sc = p2s.tile([128, 512], f32, tag="sc")
                                nc.tensor.matmul(
                                    ps_sc[:],
                                    kt_sb[:, kvh, j * 128:(j + 1) * 128],
                                    rhs_q)
                                if j >= 4 * w:
                                    nc.vector.tensor_add(ps_sc[:], ps_sc[:], masks[:, j - 4 * w, :])
                                et = ex.tile([128, 512], f32r, tag="et")
                                nc.scalar.activation(et[:], ps_sc[:],
                                                     mybir.ActivationFunctionType.Exp,
                                                     scale=SCALE)
                                st, sp = j == 0, j == njt - 1
                                nc.tensor.matmul(
                                    ps_y[:],
                                    v_sb[:, j, kvh * 128:(kvh + 1) * 128],
                                    et[:], start=st, stop=sp,
                                    skip_group_check=True)
                                nc.tensor.matmul(
                                    ps_s[:], ones[:], et[:],
                                    start=st, stop=sp, skip_group_check=True)
                            lgs = sm.tile([1, 512], f32, tag="lgs")
                            nc.scalar.activation(lgs[:], ps_s[:],
                                                 mybir.ActivationFunctionType.Ln)
                            rec = sm.tile([1, 512], f32r, tag="rec")
                            nc.scalar.activation(rec[:], lgs[:],
                                                 mybir.ActivationFunctionType.Exp,
                                                 scale=-1.0)
                            bcp = p2s.tile([128, 512], f32, tag="sc")
                            nc.tensor.matmul(bcp[:], ones_r[:], rec[:])
                            bc = sm.tile([128, 512], f32, tag="bc")
                            nc.vector.tensor_copy(bc[:], bcp[:])
                            nc.vector.tensor_mul(
                                ytn[:, hq, w * 512:(w + 1) * 512], ps_y[:], bc[:])

                # ---- Phase 3: output projection ----
                with tc.tile_pool(name="w3", bufs=1) as w3, \
                     tc.tile_pool(name="ob", bufs=4) as ob, \
                     tc.tile_pool(name="p3", bufs=4, space="PSUM") as p3:
                    wo_sb = w3.tile([128, HLOC, D], f32r, tag="wo")
                    wo_r = wo_d[:].bitcast(f32r).rearrange("(h p) c -> p h c", p=128)
                    for og in range(4):
                        nc.sync.dma_start(out=wo_sb[:, :, og * 512:(og + 1) * 512],
                                          in_=wo_r[:, :, og * 512:(og + 1) * 512])
                    for og in range(4):
                        for tcid in range(NTC):
                            ps_o = p3.tile([128, 512], f32, tag="o")
                            for h in range(HLOC):
                                nc.tensor.matmul(
                                    ps_o[:],
                                    ytn[:, h, tcid * 128:(tcid + 1) * 128],
                                    wo_sb[:, h, og * 512:(og + 1) * 512],
                                    start=(h == 0), stop=(h == HLOC - 1))
                            ot = ob.tile([128, 512], f32, tag="ot")
                            nc.vector.tensor_copy(ot[:], ps_o[:])
                            nc.scalar.dma_start(
                                out=out_d[tcid * 128:(tcid + 1) * 128, og * 512:(og + 1) * 512],
                                in_=ot[:])

    nc.compile()
    return nc


_PROGRAM = None


def _get_program():
    global _PROGRAM
    if _PROGRAM is None:
        _PROGRAM = build_program()
    return _PROGRAM


def make_in_maps(x, W_qkv, W_out):
    in_maps = []
    for c in range(8):
        b, t = c // 2, c % 2
        xt = np.ascontiguousarray(x[b].T)
        wq = np.ascontiguousarray(W_qkv[:, t * 1024:(t + 1) * 1024])
        wk = W_qkv[:, D + t * 256: D + (t + 1) * 256]
        wv = W_qkv[:, D + 512 + t * 256: D + 512 + (t + 1) * 256]
        wkv = np.ascontiguousarray(np.concatenate([wk, wv], axis=1))
        wo = np.ascontiguousarray(W_out[t * 1024:(t + 1) * 1024, :])
        in_maps.append({"xt": xt, "wq": wq, "wkv": wkv, "wo": wo})
    return in_maps


def kernel(x, W_qkv, W_out):
    from concourse.bass_utils import run_bass_kernel_spmd
    nc = _get_program()
    in_maps = make_in_maps(np.asarray(x, dtype=np.float32),
                           np.asarray(W_qkv, dtype=np.float32),
                           np.asarray(W_out, dtype=np.float32))
    res = run_bass_kernel_spmd(nc, in_maps, list(range(8)), trace=False)
    out = np.empty((B, S, D), dtype=np.float32)
    for b in range(B):
        out[b] = res.results[2 * b]["out"] + res.results[2 * b + 1]["out"]
    return out



# revision 5
# speedup vs baseline: 1.3153x; 1.3153x over previous
"""Causal self-attention (GQA + RMS-norm + RoPE) Trainium2 Bass kernel. v2

Sharding: 8 cores = 4 batches x 2 head-groups (tensor-parallel over heads).
Core c = 2*b + t handles batch b with Q heads [8t, 8t+8) and KV heads
[2t, 2t+2). Each core computes a partial output projection (its heads'
rows of W_out); the host sums the two partials per batch.

v2 changes vs v1:
 - all matmul operands bf16 (psum stays f32); qT/kT/v SBUF-resident (no
   DRAM spill round-trip between phases 1 and 2).
 - zero ACT table thrash: phase 1 uses only Square/Sqrt (one table set),
   phase 2 only Exp; reciprocals via DVE reciprocal_approx_fast.
 - phase 2 software-pipelined: scores matmul emitted one j ahead of the
   exp->PV chain so PE never stalls on ACT.
 - DMA issued from idle queues (sync + gpsimd), weight loads early.

Pipeline per core:
  P1: qkv = x @ W_shard (transposed-x input), RMS+RoPE on q/k in natural
      layout, PE-transpose q/k to [d, tok] into resident SBUF.
  P2: per 512-token query window, per head: scoresT = kT_tile.T @ qT_win,
      +tri-mask on diagonal tiles, exp (ACT, scale=hd^-0.5) -> et bf16,
      yT += v_tile.T @ et, sums += ones.T @ et; rec = 1/sums (DVE),
      broadcast via PE, ytn = yT * rec.
  P3: out = sum_h ytn_h.T @ W_out_h rows -> partial [S, D] f32.
"""
import sys, os
sys.path.insert(0, '/opt/trn_rl_repo')
import numpy as np

from concourse import bass, bacc, mybir, tile

f32 = mybir.dt.float32
f32r = mybir.dt.float32r
bf16 = mybir.dt.bfloat16

B, S, D = 4, 2048, 2048
H, HKV, HD = 16, 4, 128
HLOC = H // 2          # 8 q heads per core
KVLOC = HKV // 2       # 2 kv heads per core
SCALE = float(HD) ** -0.5
ROPE_BASE = 10000.0

NTC = S // 128         # 16 token tiles
NDT = D // 128         # 16 contraction tiles
NWIN = S // 512        # 4 query windows


def _np_bf16():
    import ml_dtypes
    return np.dtype(ml_dtypes.bfloat16)


def _rope_tables():
    inv_freq = (1.0 / (ROPE_BASE ** (np.arange(0, HD, 2, dtype=np.float32) / HD))).astype(np.float32)
    freqs = np.arange(S, dtype=np.float32)[:, None] * inv_freq[None, :]
    cos = np.cos(freqs).astype(np.float32)
    sin = np.sin(freqs).astype(np.float32)
    cos2 = np.concatenate([cos, cos], axis=1)        # [S, 128]
    sin2 = np.concatenate([sin, -sin], axis=1)       # [S, 128]
    return cos2, sin2


def _tri_masks():
    # mask[vi][p, f] = -1e30 where kv > q for scoresT diag tiles:
    # kv = 128*j + p, q = 512*w + f, vi = j - 4*w -> masked iff p + 128*vi > f
    m = np.zeros((4, 128, 512), dtype=np.float32)
    p = np.arange(128)[:, None]
    f = np.arange(512)[None, :]
    for vi in range(4):
        m[vi][(p + 128 * vi) > f] = -1e30
    return m


def _emit_rms_rope(nc, scr, psum_ap, nheads, cos1, sin1, nat_tile):
    """psum_ap: [128, nheads*128] qkv psum slice; writes RMS+RoPE result into
    nat_tile (SBUF bf16). cos1/sin1: [128, 1, 128] APs (cos dup, [sin,-sin]).

    rot(q) = q*cos2 + swap_halves(q)*sin2;  out = rot(q) * rsqrt(mean(q^2))
    rsqrt = Sqrt(HD * reciprocal(sum(q^2))) with Square/Sqrt on ACT and the
    reciprocal on DVE -> single activation table set for all of phase 1.
    """
    w = nheads * 128
    sq = scr.tile([128, w], f32, tag="sq")
    nc.scalar.activation(sq[:], psum_ap, mybir.ActivationFunctionType.Square)
    ss = scr.tile([128, nheads, 1], f32, tag="ss")
    nc.vector.tensor_reduce(
        ss[:], sq[:].rearrange("p (h f) -> p h f", h=nheads),
        axis=mybir.AxisListType.X, op=mybir.AluOpType.add)
    rr = scr.tile([128, nheads, 1], f32, tag="rr")
    nc.vector.reciprocal_approx_fast(rr[:], ss[:])
    rinv = scr.tile([128, nheads, 1, 1], f32, tag="rinv")
    nc.scalar.activation(rinv[:], rr[:], mybir.ActivationFunctionType.Sqrt,
                         scale=float(HD))

    shp = [128, nheads, 2, 64]
    p4 = psum_ap.rearrange("p (h x f) -> p h x f", h=nheads, x=2)
    p4s = p4[:, :, ::-1, :]
    cb = cos1.rearrange("p t (x f) -> p t x f", x=2).to_broadcast(shp)
    sb_ = sin1.rearrange("p t (x f) -> p t x f", x=2).to_broadcast(shp)
    rb = rinv[:].to_broadcast(shp)
    t1 = scr.tile(shp, f32, tag="t1")
    t2 = scr.tile(shp, f32, tag="t2")
    nc.vector.tensor_mul(t1[:], p4, cb)
    nc.vector.tensor_mul(t2[:], p4s, sb_)
    nc.vector.tensor_add(t1[:], t1[:], t2[:])
    nc.vector.tensor_mul(nat_tile[:].rearrange("p (h x f) -> p h x f", h=nheads, x=2),
                         t1[:], rb)


def build_program():
    cos_np, sin_np = _rope_tables()
    masks_np = _tri_masks()
    bf = _np_bf16()

    nc = bacc.Bacc(trn_type="TRN2")

    xt_d = nc.dram_tensor("xt", [D, S], bf16, kind="ExternalInput")
    wq_d = nc.dram_tensor("wq", [D, HLOC * HD], bf16, kind="ExternalInput")
    wkv_d = nc.dram_tensor("wkv", [D, 2 * KVLOC * HD], bf16, kind="ExternalInput")
    wo_d = nc.dram_tensor("wo", [HLOC * HD, D], bf16, kind="ExternalInput")
    out_d = nc.dram_tensor("out", [S, D], f32, kind="ExternalOutput")

    cos_d = nc.inline_tensor(cos_np, "cos_t")
    sin_d = nc.inline_tensor(sin_np, "sin_t")
    ident_d = nc.inline_tensor(np.eye(128, dtype=np.float32).astype(bf), "ident")
    masks_d = nc.inline_tensor(masks_np, "tri_masks")
    onescol_d = nc.inline_tensor(np.ones((128, 1), dtype=np.float32).astype(bf), "onescol")
    onesrow_d = nc.inline_tensor(np.ones((1, 128), dtype=np.float32), "onesrow")

    with tile.TileContext(nc) as tc:
        with tc.tile_pool(name="cst", bufs=1) as cst:
            cos_sb = cst.tile([128, NTC, 128], f32, tag="cos")
            sin_sb = cst.tile([128, NTC, 128], f32, tag="sin")
            ident = cst.tile([128, 128], bf16, tag="ident")
            masks = cst.tile([128, 4, 512], f32, tag="masks")
            ones = cst.tile([128, 1], bf16, tag="ones")
            ones_r = cst.tile([1, 128], f32r, tag="ones_r")
            # resident qT/kT/v (bf16)
            qt_sb = cst.tile([128, HLOC, S], bf16, tag="qt")
            kt_sb = cst.tile([128, KVLOC, NTC, 128], bf16, tag="kt")
            v_sb = cst.tile([128, NTC, KVLOC * HD], bf16, tag="v")

            nc.sync.dma_start(out=cos_sb[:], in_=cos_d[:].rearrange("(t p) f -> p t f", p=128))
            nc.sync.dma_start(out=sin_sb[:], in_=sin_d[:].rearrange("(t p) f -> p t f", p=128))
            nc.sync.dma_start(out=ident[:], in_=ident_d[:])
            nc.sync.dma_start(out=masks[:], in_=masks_d[:].rearrange("v p f -> p v f"))
            nc.sync.dma_start(out=ones[:], in_=onescol_d[:])
            nc.sync.dma_start(out=ones_r[:], in_=onesrow_d[:].bitcast(f32r))

            # ---------------- Phase 1: QKV projection ----------------
            with tc.tile_pool(name="w1", bufs=1) as w1, \
                 tc.tile_pool(name="xs", bufs=3) as xs, \
                 tc.tile_pool(name="nat", bufs=3) as nat, \
                 tc.tile_pool(name="p1a", bufs=6, space="PSUM") as p1a, \
                 tc.tile_pool(name="p1t", bufs=2, space="PSUM") as p1t:

                wq_sb = w1.tile([128, NDT, HLOC * HD], bf16, tag="wq")
                wkv_sb = w1.tile([128, NDT, 512], bf16, tag="wkv")
                wq_r = wq_d[:].rearrange("(t p) c -> p t c", p=128)
                wkv_r = wkv_d[:].rearrange("(t p) c -> p t c", p=128)
                for dt in range(NDT):
                    nc.gpsimd.dma_start(out=wkv_sb[:, dt, :], in_=wkv_r[:, dt, :])
                for dt in range(NDT):
                    nc.gpsimd.dma_start(out=wq_sb[:, dt, :], in_=wq_r[:, dt, :])

                for tcid in range(NTC):
                    xt_sb = xs.tile([128, NDT, 128], bf16, tag="xt")
                    nc.sync.dma_start(
                        out=xt_sb[:],
                        in_=xt_d[:, tcid * 128:(tcid + 1) * 128]
                            .rearrange("(t p) s -> p t s", p=128))

                    ps_q1 = p1a.tile([128, 512], f32, tag="acc")
                    ps_q2 = p1a.tile([128, 512], f32, tag="acc")
                    ps_kv = p1a.tile([128, 512], f32, tag="acc")
                    for dt in range(NDT):
                        st, sp = dt == 0, dt == NDT - 1
                        nc.tensor.matmul(ps_kv[:], xt_sb[:, dt, :], wkv_sb[:, dt, :], start=st, stop=sp)
                    for dt in range(NDT):
                        st, sp = dt == 0, dt == NDT - 1
                        lhs = xt_sb[:, dt, :]
                        nc.tensor.matmul(ps_q1[:], lhs, wq_sb[:, dt, 0:512], start=st, stop=sp)
                        nc.tensor.matmul(ps_q2[:], lhs, wq_sb[:, dt, 512:1024], start=st, stop=sp)

                    cos1 = cos_sb[:, tcid:tcid + 1, :]
                    sin1 = sin_sb[:, tcid:tcid + 1, :]

                    # q heads 0-3 / 4-7: RMS+RoPE, then PE-transpose to qT
                    for gi, ps in ((0, ps_q1), (1, ps_q2)):
                        qn = nat.tile([128, 512], bf16, tag="qn")
                        _emit_rms_rope(nc, nat, ps[:], 4, cos1, sin1, qn)
                        for hh in range(4):
                            h = gi * 4 + hh
                            tp = p1t.tile([128, 128], bf16, tag="tp")
                            nc.tensor.transpose(tp[:], qn[:, hh * 128:(hh + 1) * 128], ident[:])
                            nc.vector.tensor_copy(
                                qt_sb[:, h, tcid * 128:(tcid + 1) * 128], tp[:])

                    # k heads (cols 0:256 of kv psum)
                    kn = nat.tile([128, 256], bf16, tag="kn")
                    _emit_rms_rope(nc, nat, ps_kv[:, 0:256], 2, cos1, sin1, kn)
                    for kh in range(KVLOC):
                        tp = p1t.tile([128, 128], bf16, tag="tp")
                        nc.tensor.transpose(tp[:], kn[:, kh * 128:(kh + 1) * 128], ident[:])
                        nc.vector.tensor_copy(
                            kt_sb[:, kh, tcid, :], tp[:])

                    # v: plain copy out of psum (natural layout, bf16)
                    nc.vector.tensor_copy(v_sb[:, tcid, :], ps_kv[:, 256:512])

            # ------------- Phases 2+3 (share the ytn resident) -------------
            with tc.tile_pool(name="ytp", bufs=1) as ytp:
                ytn = ytp.tile([128, HLOC, S], bf16, tag="ytn")

                with tc.tile_pool(name="w3", bufs=1) as w3:
                    # W_out loaded during phase 2 (needed at phase 3 start)
                    wo_sb = w3.tile([128, HLOC, D], bf16, tag="wo")
                    wo_r = wo_d[:].rearrange("(h p) c -> p h c", p=128)
                    for og in range(4):
                        nc.gpsimd.dma_start(out=wo_sb[:, :, og * 512:(og + 1) * 512],
                                            in_=wo_r[:, :, og * 512:(og + 1) * 512])

                    # ---- Phase 2: attention ----
                    with tc.tile_pool(name="ex", bufs=6) as ex, \
                         tc.tile_pool(name="sm", bufs=4) as sm, \
                         tc.tile_pool(name="p2s", bufs=4, space="PSUM") as p2s, \
                         tc.tile_pool(name="p2y", bufs=2, space="PSUM") as p2y, \
                         tc.tile_pool(name="p2n", bufs=2, space="PSUM") as p2n:

                        for w in range(NWIN):
                            njt = 4 * w + 4
                            for hq in range(HLOC):
                                kvh = hq // 4
                                ps_y = p2y.tile([128, 512], f32, tag="y")
                                ps_s = p2n.tile([1, 512], f32, tag="s")
                                rhs_q = qt_sb[:, hq, w * 512:(w + 1) * 512]
                                sc_tiles = {}

                                def emit_sc(j):
                                    ps_sc = p2s.tile([128, 512], f32, tag="sc")
                                    nc.tensor.matmul(
                                        ps_sc[:], kt_sb[:, kvh, j, :], rhs_q)
                                    if j >= 4 * w:
                                        nc.vector.tensor_add(ps_sc[:], ps_sc[:],
                                                             masks[:, j - 4 * w, :])
                                    sc_tiles[j] = ps_sc

                                emit_sc(0)
                                for j in range(njt):
                                    if j + 1 < njt:
                                        emit_sc(j + 1)
                                    ps_sc = sc_tiles.pop(j)
                                    et = ex.tile([128, 512], bf16, tag="et")
                                    nc.scalar.activation(et[:], ps_sc[:],
                                                         mybir.ActivationFunctionType.Exp,
                                                         scale=SCALE)
                                    st, sp = j == 0, j == njt - 1
                                    nc.tensor.matmul(
                                        ps_y[:],
                                        v_sb[:, j, kvh * 128:(kvh + 1) * 128],
                                        et[:], start=st, stop=sp,
                                        skip_group_check=True)
                                    nc.tensor.matmul(
                                        ps_s[:], ones[:], et[:],
                                        start=st, stop=sp, skip_group_check=True)

                                rec = sm.tile([1, 512], f32, tag="rec")
                                nc.vector.reciprocal_approx_fast(rec[:], ps_s[:])
                                recr = sm.tile([1, 512], f32r, tag="recr")
                                nc.vector.tensor_copy(recr[:], rec[:])
                                bcp = p2s.tile([128, 512], f32, tag="sc")
                                nc.tensor.matmul(bcp[:], ones_r[:], recr[:])
                                bc = sm.tile([128, 512], f32, tag="bc")
                                nc.vector.tensor_copy(bc[:], bcp[:])
                                nc.vector.tensor_mul(
                                    ytn[:, hq, w * 512:(w + 1) * 512], ps_y[:], bc[:])

                    # ---- Phase 3: output projection ----
                    with tc.tile_pool(name="ob", bufs=4) as ob, \
                         tc.tile_pool(name="p3", bufs=4, space="PSUM") as p3:
                        for tcid in range(NTC):
                            for og in range(4):
                                ps_o = p3.tile([128, 512], f32, tag="o")
                                for h in range(HLOC):
                                    nc.tensor.matmul(
                                        ps_o[:],
                                        ytn[:, h, tcid * 128:(tcid + 1) * 128],
                                        wo_sb[:, h, og * 512:(og + 1) * 512],
                                        start=(h == 0), stop=(h == HLOC - 1))
                                ot = ob.tile([128, 512], f32, tag="ot")
                                nc.scalar.activation(ot[:], ps_o[:],
                                                     mybir.ActivationFunctionType.Copy)
                                nc.gpsimd.dma_start(
                                    out=out_d[tcid * 128:(tcid + 1) * 128, og * 512:(og + 1) * 512],
                                    in_=ot[:])

    nc.compile()
    return nc


_PROGRAM = None


def _get_program():
    global _PROGRAM
    if _PROGRAM is None:
        _PROGRAM = build_program()
    return _PROGRAM


def make_in_maps(x, W_qkv, W_out):
    bf = _np_bf16()
    x = np.asarray(x, dtype=np.float32)
    W_qkv = np.asarray(W_qkv, dtype=np.float32)
    W_out = np.asarray(W_out, dtype=np.float32)
    in_maps = []
    for c in range(8):
        b, t = c // 2, c % 2
        xt = np.ascontiguousarray(x[b].T).astype(bf)
        wq = np.ascontiguousarray(W_qkv[:, t * 1024:(t + 1) * 1024]).astype(bf)
        wk = W_qkv[:, D + t * 256: D + (t + 1) * 256]
        wv = W_qkv[:, D + 512 + t * 256: D + 512 + (t + 1) * 256]
        wkv = np.ascontiguousarray(np.concatenate([wk, wv], axis=1)).astype(bf)
        wo = np.ascontiguousarray(W_out[t * 1024:(t + 1) * 1024, :]).astype(bf)
        in_maps.append({"xt": xt, "wq": wq, "wkv": wkv, "wo": wo})
    return in_maps


def kernel(x, W_qkv, W_out):
    from concourse.bass_utils import run_bass_kernel_spmd
    nc = _get_program()
    in_maps = make_in_maps(x, W_qkv, W_out)
    res = run_bass_kernel_spmd(nc, in_maps, list(range(8)), trace=False)
    out = np.empty((B, S, D), dtype=np.float32)
    for b in range(B):
        out[b] = res.results[2 * b]["out"] + res.results[2 * b + 1]["out"]
    return out


# revision 12
# speedup vs baseline: 1.6808x; 1.2779x over previous
"""Causal self-attention (GQA + RMS-norm + RoPE) Trainium2 Bass kernel. v2

Sharding: 8 cores = 4 batches x 2 head-groups (tensor-parallel over heads).
Core c = 2*b + t handles batch b with Q heads [8t, 8t+8) and KV heads
[2t, 2t+2). Each core computes a partial output projection (its heads'
rows of W_out); the host sums the two partials per batch.

v2 changes vs v1:
 - all matmul operands bf16 (psum stays f32); qT/kT/v SBUF-resident (no
   DRAM spill round-trip between phases 1 and 2).
 - zero ACT table thrash: phase 1 uses only Square/Sqrt (one table set),
   phase 2 only Exp; reciprocals via DVE reciprocal_approx_fast.
 - phase 2 software-pipelined: scores matmul emitted one j ahead of the
   exp->PV chain so PE never stalls on ACT.
 - DMA issued from idle queues (sync + gpsimd), weight loads early.

Pipeline per core:
  P1: qkv = x @ W_shard (transposed-x input), RMS+RoPE on q/k in natural
      layout, PE-transpose q/k to [d, tok] into resident SBUF.
  P2: per 512-token query window, per head: scoresT = kT_tile.T @ qT_win,
      +tri-mask on diagonal tiles, exp (ACT, scale=hd^-0.5) -> et bf16,
      yT += v_tile.T @ et, sums += ones.T @ et; rec = 1/sums (DVE),
      broadcast via PE, ytn = yT * rec.
  P3: out = sum_h ytn_h.T @ W_out_h rows -> partial [S, D] f32.
"""
import sys, os
sys.path.insert(0, '/opt/trn_rl_repo')
import numpy as np

from concourse import bass, bacc, mybir, tile

f32 = mybir.dt.float32
f32r = mybir.dt.float32r
bf16 = mybir.dt.bfloat16

B, S, D = 4, 2048, 2048
H, HKV, HD = 16, 4, 128
HLOC = H // 2          # 8 q heads per core
KVLOC = HKV // 2       # 2 kv heads per core
SCALE = float(HD) ** -0.5
ROPE_BASE = 10000.0

NTC = S // 128         # 16 token tiles
NDT = D // 128         # 16 contraction tiles
NWIN = S // 512        # 4 query windows


def _np_bf16():
    import ml_dtypes
    return np.dtype(ml_dtypes.bfloat16)


def _rope_tables():
    inv_freq = (1.0 / (ROPE_BASE ** (np.arange(0, HD, 2, dtype=np.float32) / HD))).astype(np.float32)
    freqs = np.arange(S, dtype=np.float32)[:, None] * inv_freq[None, :]
    cos = np.cos(freqs).astype(np.float32)
    sin = np.sin(freqs).astype(np.float32)
    cos2 = np.concatenate([cos, cos], axis=1)        # [S, 128]
    sin2 = np.concatenate([sin, -sin], axis=1)       # [S, 128]
    return cos2, sin2


def _tri_masks():
    # mask[vi][p, f] = -1e30 where kv > q for scoresT diag tiles:
    # kv = 128*j + p, q = 512*w + f, vi = j - 4*w -> masked iff p + 128*vi > f
    m = np.zeros((4, 128, 512), dtype=np.float32)
    p = np.arange(128)[:, None]
    f = np.arange(512)[None, :]
    for vi in range(4):
        m[vi][(p + 128 * vi) > f] = -1e30
    return m


def _emit_rms_rope(nc, scr, psum_ap, nheads, cos1, sin1, nat_tile):
    """psum_ap: [128, nheads*128] qkv psum slice; writes RMS+RoPE result into
    nat_tile (SBUF bf16). cos1/sin1: [128, 1, 128] APs (cos dup, [sin,-sin]).

    rot(q) = q*cos2 + swap_halves(q)*sin2;  out = rot(q) * rsqrt(mean(q^2))
    rsqrt = Sqrt(HD * reciprocal(sum(q^2))) with Square/Sqrt on ACT and the
    reciprocal on DVE -> single activation table set for all of phase 1.
    """
    w = nheads * 128
    sq = scr.tile([128, w], f32, tag="sq")
    nc.scalar.activation(sq[:], psum_ap, mybir.ActivationFunctionType.Square)
    ss = scr.tile([128, nheads, 1], f32, tag="ss")
    nc.vector.tensor_reduce(
        ss[:], sq[:].rearrange("p (h f) -> p h f", h=nheads),
        axis=mybir.AxisListType.X, op=mybir.AluOpType.add)
    rr = scr.tile([128, nheads, 1], f32, tag="rr")
    nc.vector.reciprocal_approx_fast(rr[:], ss[:])
    rinv = scr.tile([128, nheads, 1, 1], f32, tag="rinv")
    nc.scalar.activation(rinv[:], rr[:], mybir.ActivationFunctionType.Sqrt,
                         scale=float(HD))

    shp = [128, nheads, 2, 64]
    p4 = psum_ap.rearrange("p (h x f) -> p h x f", h=nheads, x=2)
    p4s = p4[:, :, ::-1, :]
    cb = cos1.rearrange("p t (x f) -> p t x f", x=2).to_broadcast(shp)
    sb_ = sin1.rearrange("p t (x f) -> p t x f", x=2).to_broadcast(shp)
    rb = rinv[:].to_broadcast(shp)
    t1 = scr.tile(shp, f32, tag="t1")
    t2 = scr.tile(shp, f32, tag="t2")
    nc.vector.tensor_mul(t1[:], p4, cb)
    nc.vector.tensor_mul(t2[:], p4s, sb_)
    nc.vector.tensor_add(t1[:], t1[:], t2[:])
    nc.vector.tensor_mul(nat_tile[:].rearrange("p (h x f) -> p h x f", h=nheads, x=2),
                         t1[:], rb)


def build_program():
    cos_np, sin_np = _rope_tables()
    masks_np = _tri_masks()
    bf = _np_bf16()

    nc = bacc.Bacc(trn_type="TRN2")

    xt_d = nc.dram_tensor("xt", [D, S], bf16, kind="ExternalInput")
    wq_d = nc.dram_tensor("wq", [D, HLOC * HD], bf16, kind="ExternalInput")
    wkv_d = nc.dram_tensor("wkv", [D, 2 * KVLOC * HD], bf16, kind="ExternalInput")
    wo_d = nc.dram_tensor("wo", [HLOC * HD, D], bf16, kind="ExternalInput")
    out_d = nc.dram_tensor("out", [S, D], f32, kind="ExternalOutput")

    cos_d = nc.inline_tensor(cos_np, "cos_t")
    sin_d = nc.inline_tensor(sin_np, "sin_t")
    ident_d = nc.inline_tensor(np.eye(128, dtype=np.float32).astype(bf), "ident")
    masks_d = nc.inline_tensor(masks_np, "tri_masks")
    onescol_d = nc.inline_tensor(np.ones((128, 128), dtype=np.float32).astype(bf), "onescol")

    with tile.TileContext(nc) as tc:
        with tc.tile_pool(name="cst", bufs=1) as cst:
            cos_sb = cst.tile([128, NTC, 128], f32, tag="cos")
            sin_sb = cst.tile([128, NTC, 128], f32, tag="sin")
            ident = cst.tile([128, 128], bf16, tag="ident")
            masks = cst.tile([128, 4, 512], f32, tag="masks")
            ones = cst.tile([128, 128], bf16, tag="ones")
            # resident qT/kT/v (bf16)
            qt_sb = cst.tile([128, HLOC, S], bf16, tag="qt")
            kt_sb = cst.tile([128, KVLOC, NTC, 128], bf16, tag="kt")
            v_sb = cst.tile([128, NTC, KVLOC * HD], bf16, tag="v")

            # constants on the scalar queue so the sync queue leads with x
            # tile 0
            nc.scalar.dma_start(out=ident[:], in_=ident_d[:])
            nc.scalar.dma_start(out=ones[:], in_=onescol_d[:])
            nc.scalar.dma_start(out=cos_sb[:], in_=cos_d[:].rearrange("(t p) f -> p t f", p=128))
            nc.scalar.dma_start(out=sin_sb[:], in_=sin_d[:].rearrange("(t p) f -> p t f", p=128))
            nc.scalar.dma_start(out=masks[:], in_=masks_d[:].rearrange("v p f -> p v f"))

            # ---------------- Phase 1: QKV projection ----------------
            with tc.tile_pool(name="w1", bufs=1) as w1, \
                 tc.tile_pool(name="xs", bufs=3) as xs, \
                 tc.tile_pool(name="nat", bufs=3) as nat, \
                 tc.tile_pool(name="p1q", bufs=2, space="PSUM") as p1q, \
                 tc.tile_pool(name="p1k", bufs=2, space="PSUM") as p1k, \
                 tc.tile_pool(name="p1t", bufs=2, space="PSUM") as p1t:

                # x tile 0 leads the sync queue
                xt_tiles = []
                xt_sb0 = xs.tile([128, NDT, 128], bf16, tag="xt")
                nc.sync.dma_start(
                    out=xt_sb0[:],
                    in_=xt_d[:, 0:128].rearrange("(t p) s -> p t s", p=128))
                xt_tiles.append(xt_sb0)

                wq_sb = w1.tile([128, NDT, HLOC * HD], bf16, tag="wq")
                wkv_sb = w1.tile([128, NDT, 512], bf16, tag="wkv")
                wq_r = wq_d[:].rearrange("(t p) c -> p t c", p=128)
                wkv_r = wkv_d[:].rearrange("(t p) c -> p t c", p=128)
                for dt in range(NDT):
                    nc.gpsimd.dma_start(out=wkv_sb[:, dt, :], in_=wkv_r[:, dt, :])
                    nc.scalar.dma_start(out=wq_sb[:, dt, 0:512], in_=wq_r[:, dt, 0:512])
                    nc.sync.dma_start(out=wq_sb[:, dt, 512:1024], in_=wq_r[:, dt, 512:1024])

                for tcid in range(NTC):
                    if tcid == 0:
                        xt_sb = xt_tiles[0]
                    else:
                        xt_sb = xs.tile([128, NDT, 128], bf16, tag="xt")
                        nc.sync.dma_start(
                            out=xt_sb[:],
                            in_=xt_d[:, tcid * 128:(tcid + 1) * 128]
                                .rearrange("(t p) s -> p t s", p=128))

                    ps_q = p1q.tile([128, 1024], f32, tag="qacc")
                    ps_kv = p1k.tile([128, 512], f32, tag="kvacc")
                    for dt in range(NDT):
                        st, sp = dt == 0, dt == NDT - 1
                        nc.tensor.matmul(ps_kv[:], xt_sb[:, dt, :], wkv_sb[:, dt, :], start=st, stop=sp)
                    for dt in range(NDT):
                        st, sp = dt == 0, dt == NDT - 1
                        lhs = xt_sb[:, dt, :]
                        nc.tensor.matmul(ps_q[:, 0:512], lhs, wq_sb[:, dt, 0:512], start=st, stop=sp)
                        nc.tensor.matmul(ps_q[:, 512:1024], lhs, wq_sb[:, dt, 512:1024], start=st, stop=sp)

                    cos1 = cos_sb[:, tcid:tcid + 1, :]
                    sin1 = sin_sb[:, tcid:tcid + 1, :]

                    # all 8 q heads at once: RMS+RoPE, then PE-transpose to qT
                    qn = nat.tile([128, 1024], bf16, tag="qn")
                    _emit_rms_rope(nc, nat, ps_q[:], 8, cos1, sin1, qn)
                    for h in range(HLOC):
                        tp = p1t.tile([128, 128], bf16, tag="tp")
                        nc.tensor.transpose(tp[:], qn[:, h * 128:(h + 1) * 128], ident[:])
                        nc.vector.tensor_copy(
                            qt_sb[:, h, tcid * 128:(tcid + 1) * 128], tp[:])

                    # k heads (cols 0:256 of kv psum)
                    kn = nat.tile([128, 256], bf16, tag="kn")
                    _emit_rms_rope(nc, nat, ps_kv[:, 0:256], 2, cos1, sin1, kn)
                    for kh in range(KVLOC):
                        tp = p1t.tile([128, 128], bf16, tag="tp")
                        nc.tensor.transpose(tp[:], kn[:, kh * 128:(kh + 1) * 128], ident[:])
                        nc.vector.tensor_copy(
                            kt_sb[:, kh, tcid, :], tp[:])

                    # v: plain copy out of psum (natural layout, bf16)
                    nc.vector.tensor_copy(v_sb[:, tcid, :], ps_kv[:, 256:512])

            # ------------- Phases 2+3 (share the ytn resident) -------------
            with tc.tile_pool(name="ytp", bufs=1) as ytp:
                ytn = ytp.tile([128, HLOC, S], bf16, tag="ytn")

                with tc.tile_pool(name="w3", bufs=1) as w3:
                    # W_out loaded during phase 2 (needed at phase 3 start)
                    wo_sb = w3.tile([128, HLOC, D], bf16, tag="wo")
                    wo_r = wo_d[:].rearrange("(h p) c -> p h c", p=128)
                    for og in range(4):
                        nc.gpsimd.dma_start(out=wo_sb[:, :, og * 512:(og + 1) * 512],
                                            in_=wo_r[:, :, og * 512:(og + 1) * 512])

                    # ---- Phase 2: attention ----
                    with tc.tile_pool(name="ex", bufs=6) as ex, \
                         tc.tile_pool(name="sm", bufs=4) as sm, \
                         tc.tile_pool(name="p2s", bufs=4, space="PSUM") as p2s, \
                         tc.tile_pool(name="p2y", bufs=2, space="PSUM") as p2y, \
                         tc.tile_pool(name="p2n", bufs=2, space="PSUM") as p2n:

                        for w in range(NWIN):
                            njt = 4 * w + 4
                            for hq in range(HLOC):
                                kvh = hq // 4
                                ps_y = p2y.tile([128, 512], f32, tag="y")
                                # sums broadcast to all 128 partitions by the
                                # all-ones [128,128] stationary
                                ps_s = p2n.tile([128, 512], f32, tag="s")
                                rhs_q = qt_sb[:, hq, w * 512:(w + 1) * 512]
                                sc_tiles = {}

                                def emit_sc(j):
                                    ps_sc = p2s.tile([128, 512], f32, tag="sc")
                                    nc.tensor.matmul(
                                        ps_sc[:], kt_sb[:, kvh, j, :], rhs_q)
                                    if j >= 4 * w:
                                        nc.vector.tensor_add(ps_sc[:], ps_sc[:],
                                                             masks[:, j - 4 * w, :])
                                    sc_tiles[j] = ps_sc

                                emit_sc(0)
                                for j in range(njt):
                                    if j + 1 < njt:
                                        emit_sc(j + 1)
                                    ps_sc = sc_tiles.pop(j)
                                    et = ex.tile([128, 512], bf16, tag="et")
                                    nc.scalar.activation(et[:], ps_sc[:],
                                                         mybir.ActivationFunctionType.Exp,
                                                         scale=SCALE)
                                    st, sp = j == 0, j == njt - 1
                                    nc.tensor.matmul(
                                        ps_y[:],
                                        v_sb[:, j, kvh * 128:(kvh + 1) * 128],
                                        et[:], start=st, stop=sp,
                                        skip_group_check=True)
                                    nc.tensor.matmul(
                                        ps_s[:], ones[:], et[:],
                                        start=st, stop=sp, skip_group_check=True)

                                rec = sm.tile([128, 512], f32, tag="rec")
                                nc.vector.reciprocal_approx_fast(rec[:], ps_s[:])
                                nc.vector.tensor_mul(
                                    ytn[:, hq, w * 512:(w + 1) * 512], ps_y[:], rec[:])

                    # ---- Phase 3: output projection ----
                    with tc.tile_pool(name="ob", bufs=4) as ob, \
                         tc.tile_pool(name="p3", bufs=4, space="PSUM") as p3:
                        for tcid in range(NTC):
                            for og in range(4):
                                ps_o = p3.tile([128, 512], f32, tag="o")
                                for h in range(HLOC):
                                    nc.tensor.matmul(
                                        ps_o[:],
                                        ytn[:, h, tcid * 128:(tcid + 1) * 128],
                                        wo_sb[:, h, og * 512:(og + 1) * 512],
                                        start=(h == 0), stop=(h == HLOC - 1))
                                ot = ob.tile([128, 512], f32, tag="ot")
                                nc.scalar.activation(ot[:], ps_o[:],
                                                     mybir.ActivationFunctionType.Copy)
                                nc.gpsimd.dma_start(
                                    out=out_d[tcid * 128:(tcid + 1) * 128, og * 512:(og + 1) * 512],
                                    in_=ot[:])

    nc.compile()
    return nc


_PROGRAM = None


def _get_program():
    global _PROGRAM
    if _PROGRAM is None:
        _PROGRAM = build_program()
    return _PROGRAM


def make_in_maps(x, W_qkv, W_out):
    bf = _np_bf16()
    x = np.asarray(x, dtype=np.float32)
    W_qkv = np.asarray(W_qkv, dtype=np.float32)
    W_out = np.asarray(W_out, dtype=np.float32)
    in_maps = []
    for c in range(8):
        b, t = c // 2, c % 2
        xt = np.ascontiguousarray(x[b].T).astype(bf)
        wq = np.ascontiguousarray(W_qkv[:, t * 1024:(t + 1) * 1024]).astype(bf)
        wk = W_qkv[:, D + t * 256: D + (t + 1) * 256]
        wv = W_qkv[:, D + 512 + t * 256: D + 512 + (t + 1) * 256]
        wkv = np.ascontiguousarray(np.concatenate([wk, wv], axis=1)).astype(bf)
        wo = np.ascontiguousarray(W_out[t * 1024:(t + 1) * 1024, :]).astype(bf)
        in_maps.append({"xt": xt, "wq": wq, "wkv": wkv, "wo": wo})
    return in_maps


def kernel(x, W_qkv, W_out):
    from concourse.bass_utils import run_bass_kernel_spmd
    nc = _get_program()
    in_maps = make_in_maps(x, W_qkv, W_out)
    res = run_bass_kernel_spmd(nc, in_maps, list(range(8)), trace=False)
    out = np.empty((B, S, D), dtype=np.float32)
    for b in range(B):
        out[b] = res.results[2 * b]["out"] + res.results[2 * b + 1]["out"]
    return out


# revision 15
# speedup vs baseline: 1.6893x; 1.0051x over previous
"""Causal self-attention (GQA + RMS-norm + RoPE) Trainium2 Bass kernel. v2

Sharding: 8 cores = 4 batches x 2 head-groups (tensor-parallel over heads).
Core c = 2*b + t handles batch b with Q heads [8t, 8t+8) and KV heads
[2t, 2t+2). Each core computes a partial output projection (its heads'
rows of W_out); the host sums the two partials per batch.

v2 changes vs v1:
 - all matmul operands bf16 (psum stays f32); qT/kT/v SBUF-resident (no
   DRAM spill round-trip between phases 1 and 2).
 - zero ACT table thrash: phase 1 uses only Square/Sqrt (one table set),
   phase 2 only Exp; reciprocals via DVE reciprocal_approx_fast.
 - phase 2 software-pipelined: scores matmul emitted one j ahead of the
   exp->PV chain so PE never stalls on ACT.
 - DMA issued from idle queues (sync + gpsimd), weight loads early.

Pipeline per core:
  P1: qkv = x @ W_shard (transposed-x input), RMS+RoPE on q/k in natural
      layout, PE-transpose q/k to [d, tok] into resident SBUF.
  P2: per 512-token query window, per head: scoresT = kT_tile.T @ qT_win,
      +tri-mask on diagonal tiles, exp (ACT, scale=hd^-0.5) -> et bf16,
      yT += v_tile.T @ et, sums += ones.T @ et; rec = 1/sums (DVE),
      broadcast via PE, ytn = yT * rec.
  P3: out = sum_h ytn_h.T @ W_out_h rows -> partial [S, D] f32.
"""
import sys, os
sys.path.insert(0, '/opt/trn_rl_repo')
import numpy as np

from concourse import bass, bacc, mybir, tile

f32 = mybir.dt.float32
f32r = mybir.dt.float32r
bf16 = mybir.dt.bfloat16

B, S, D = 4, 2048, 2048
H, HKV, HD = 16, 4, 128
HLOC = H // 2          # 8 q heads per core
KVLOC = HKV // 2       # 2 kv heads per core
SCALE = float(HD) ** -0.5
ROPE_BASE = 10000.0

NTC = S // 128         # 16 token tiles
NDT = D // 128         # 16 contraction tiles
NWIN = S // 512        # 4 query windows


def _np_bf16():
    import ml_dtypes
    return np.dtype(ml_dtypes.bfloat16)


def _rope_tables():
    inv_freq = (1.0 / (ROPE_BASE ** (np.arange(0, HD, 2, dtype=np.float32) / HD))).astype(np.float32)
    freqs = np.arange(S, dtype=np.float32)[:, None] * inv_freq[None, :]
    cos = np.cos(freqs).astype(np.float32)
    sin = np.sin(freqs).astype(np.float32)
    cos2 = np.concatenate([cos, cos], axis=1)        # [S, 128]
    sin2 = np.concatenate([sin, -sin], axis=1)       # [S, 128]
    return cos2, sin2


def _tri_masks():
    # mask[vi][p, f] = -1e30 where kv > q for scoresT diag tiles:
    # kv = 128*j + p, q = 512*w + f, vi = j - 4*w -> masked iff p + 128*vi > f
    m = np.zeros((4, 128, 512), dtype=np.float32)
    p = np.arange(128)[:, None]
    f = np.arange(512)[None, :]
    for vi in range(4):
        m[vi][(p + 128 * vi) > f] = -1e30
    return m


def _emit_rms_rope(nc, scr, psum_ap, nheads, cos1, sin1, nat_tile):
    """psum_ap: [128, nheads*128] qkv psum slice; writes RMS+RoPE result into
    nat_tile (SBUF bf16). cos1/sin1: [128, 1, 128] APs (cos dup, [sin,-sin]).

    rot(q) = q*cos2 + swap_halves(q)*sin2;  out = rot(q) * rsqrt(mean(q^2))
    rsqrt = Sqrt(HD * reciprocal(sum(q^2))) with Square/Sqrt on ACT and the
    reciprocal on DVE -> single activation table set for all of phase 1.
    The psum is drained immediately by the two ACT reads (Square + Copy) so
    its banks recycle fast; the DVE RoPE chain reads the SBUF copy.
    """
    w = nheads * 128
    sq = scr.tile([128, w], f32, tag="sq")
    nc.scalar.activation(sq[:], psum_ap, mybir.ActivationFunctionType.Square)
    qf = scr.tile([128, w], f32, tag="qf")
    nc.scalar.activation(qf[:], psum_ap, mybir.ActivationFunctionType.Copy)
    ss = scr.tile([128, nheads, 1], f32, tag="ss")
    nc.vector.tensor_reduce(
        ss[:], sq[:].rearrange("p (h f) -> p h f", h=nheads),
        axis=mybir.AxisListType.X, op=mybir.AluOpType.add)
    rr = scr.tile([128, nheads, 1], f32, tag="rr")
    nc.vector.reciprocal_approx_fast(rr[:], ss[:])
    rinv = scr.tile([128, nheads, 1, 1], f32, tag="rinv")
    nc.scalar.activation(rinv[:], rr[:], mybir.ActivationFunctionType.Sqrt,
                         scale=float(HD))

    shp = [128, nheads, 2, 64]
    p4 = qf[:].rearrange("p (h x f) -> p h x f", h=nheads, x=2)
    p4s = p4[:, :, ::-1, :]
    cb = cos1.rearrange("p t (x f) -> p t x f", x=2).to_broadcast(shp)
    sb_ = sin1.rearrange("p t (x f) -> p t x f", x=2).to_broadcast(shp)
    rb = rinv[:].to_broadcast(shp)
    t1 = scr.tile(shp, f32, tag="t1")
    t2 = scr.tile(shp, f32, tag="t2")
    nc.vector.tensor_mul(t1[:], p4, cb)
    nc.vector.tensor_mul(t2[:], p4s, sb_)
    nc.vector.tensor_add(t1[:], t1[:], t2[:])
    nc.vector.tensor_mul(nat_tile[:].rearrange("p (h x f) -> p h x f", h=nheads, x=2),
                         t1[:], rb)


def build_program():
    cos_np, sin_np = _rope_tables()
    masks_np = _tri_masks()
    bf = _np_bf16()

    nc = bacc.Bacc(trn_type="TRN2")

    xt_d = nc.dram_tensor("xt", [D, S], bf16, kind="ExternalInput")
    wq_d = nc.dram_tensor("wq", [D, HLOC * HD], bf16, kind="ExternalInput")
    wkv_d = nc.dram_tensor("wkv", [D, 2 * KVLOC * HD], bf16, kind="ExternalInput")
    wo_d = nc.dram_tensor("wo", [HLOC * HD, D], bf16, kind="ExternalInput")
    out_d = nc.dram_tensor("out", [S, D], f32, kind="ExternalOutput")

    cos_d = nc.inline_tensor(cos_np, "cos_t")
    sin_d = nc.inline_tensor(sin_np, "sin_t")
    ident_d = nc.inline_tensor(np.eye(128, dtype=np.float32).astype(bf), "ident")
    masks_d = nc.inline_tensor(masks_np, "tri_masks")
    onescol_d = nc.inline_tensor(np.ones((128, 128), dtype=np.float32).astype(bf), "onescol")

    with tile.TileContext(nc) as tc:
        with tc.tile_pool(name="cst", bufs=1) as cst:
            cos_sb = cst.tile([128, NTC, 128], f32, tag="cos")
            sin_sb = cst.tile([128, NTC, 128], f32, tag="sin")
            ident = cst.tile([128, 128], bf16, tag="ident")
            masks = cst.tile([128, 4, 512], f32, tag="masks")
            ones = cst.tile([128, 128], bf16, tag="ones")
            # resident qT/kT/v (bf16)
            qt_sb = cst.tile([128, HLOC, S], bf16, tag="qt")
            kt_sb = cst.tile([128, KVLOC, NTC, 128], bf16, tag="kt")
            v_sb = cst.tile([128, NTC, KVLOC * HD], bf16, tag="v")

            # constants on the scalar queue so the sync queue leads with x
            # tile 0
            nc.scalar.dma_start(out=ident[:], in_=ident_d[:])
            nc.scalar.dma_start(out=ones[:], in_=onescol_d[:])
            nc.scalar.dma_start(out=cos_sb[:], in_=cos_d[:].rearrange("(t p) f -> p t f", p=128))
            nc.scalar.dma_start(out=sin_sb[:], in_=sin_d[:].rearrange("(t p) f -> p t f", p=128))
            nc.scalar.dma_start(out=masks[:], in_=masks_d[:].rearrange("v p f -> p v f"))

            # ---------------- Phase 1: QKV projection ----------------
            with tc.tile_pool(name="w1", bufs=1) as w1, \
                 tc.tile_pool(name="xs", bufs=3) as xs, \
                 tc.tile_pool(name="nat", bufs=3) as nat, \
                 tc.tile_pool(name="p1q", bufs=2, space="PSUM") as p1q, \
                 tc.tile_pool(name="p1k", bufs=2, space="PSUM") as p1k, \
                 tc.tile_pool(name="p1t", bufs=2, space="PSUM") as p1t:

                # x tile 0 leads the sync queue
                xt_tiles = []
                xt_sb0 = xs.tile([128, NDT, 128], bf16, tag="xt")
                nc.sync.dma_start(
                    out=xt_sb0[:],
                    in_=xt_d[:, 0:128].rearrange("(t p) s -> p t s", p=128))
                xt_tiles.append(xt_sb0)

                wq_sb = w1.tile([128, NDT, HLOC * HD], bf16, tag="wq")
                wkv_sb = w1.tile([128, NDT, 512], bf16, tag="wkv")
                wq_r = wq_d[:].rearrange("(t p) c -> p t c", p=128)
                wkv_r = wkv_d[:].rearrange("(t p) c -> p t c", p=128)
                for dt in range(NDT):
                    nc.gpsimd.dma_start(out=wkv_sb[:, dt, :], in_=wkv_r[:, dt, :])
                    nc.scalar.dma_start(out=wq_sb[:, dt, 0:512], in_=wq_r[:, dt, 0:512])
                    nc.sync.dma_start(out=wq_sb[:, dt, 512:1024], in_=wq_r[:, dt, 512:1024])

                for tcid in range(NTC):
                    if tcid == 0:
                        xt_sb = xt_tiles[0]
                    else:
                        xt_sb = xs.tile([128, NDT, 128], bf16, tag="xt")
                        nc.sync.dma_start(
                            out=xt_sb[:],
                            in_=xt_d[:, tcid * 128:(tcid + 1) * 128]
                                .rearrange("(t p) s -> p t s", p=128))

                    ps_q = p1q.tile([128, 1024], f32, tag="qacc")
                    ps_kv = p1k.tile([128, 512], f32, tag="kvacc")
                    for dt in range(NDT):
                        st, sp = dt == 0, dt == NDT - 1
                        nc.tensor.matmul(ps_kv[:], xt_sb[:, dt, :], wkv_sb[:, dt, :], start=st, stop=sp)
                    for dt in range(NDT):
                        st, sp = dt == 0, dt == NDT - 1
                        lhs = xt_sb[:, dt, :]
                        nc.tensor.matmul(ps_q[:, 0:512], lhs, wq_sb[:, dt, 0:512], start=st, stop=sp)
                        nc.tensor.matmul(ps_q[:, 512:1024], lhs, wq_sb[:, dt, 512:1024], start=st, stop=sp)

                    cos1 = cos_sb[:, tcid:tcid + 1, :]
                    sin1 = sin_sb[:, tcid:tcid + 1, :]

                    # all 8 q heads at once: RMS+RoPE, then PE-transpose to qT
                    qn = nat.tile([128, 1024], bf16, tag="qn")
                    _emit_rms_rope(nc, nat, ps_q[:], 8, cos1, sin1, qn)
                    for h in range(HLOC):
                        tp = p1t.tile([128, 128], bf16, tag="tp")
                        nc.tensor.transpose(tp[:], qn[:, h * 128:(h + 1) * 128], ident[:])
                        nc.vector.tensor_copy(
                            qt_sb[:, h, tcid * 128:(tcid + 1) * 128], tp[:])

                    # k heads (cols 0:256 of kv psum)
                    kn = nat.tile([128, 256], bf16, tag="kn")
                    _emit_rms_rope(nc, nat, ps_kv[:, 0:256], 2, cos1, sin1, kn)
                    for kh in range(KVLOC):
                        tp = p1t.tile([128, 128], bf16, tag="tp")
                        nc.tensor.transpose(tp[:], kn[:, kh * 128:(kh + 1) * 128], ident[:])
                        nc.vector.tensor_copy(
                            kt_sb[:, kh, tcid, :], tp[:])

                    # v: copy out of psum (natural layout, bf16) on ACT so the
                    # kv psum drains without waiting on the DVE backlog
                    nc.scalar.activation(v_sb[:, tcid, :], ps_kv[:, 256:512],
                                         mybir.ActivationFunctionType.Copy)

            # ------------- Phases 2+3 (share the ytn resident) -------------
            with tc.tile_pool(name="ytp", bufs=1) as ytp:
                ytn = ytp.tile([128, HLOC, S], bf16, tag="ytn")

                with tc.tile_pool(name="w3", bufs=1) as w3:
                    # W_out loaded during phase 2 (needed at phase 3 start)
                    wo_sb = w3.tile([128, HLOC, D], bf16, tag="wo")
                    wo_r = wo_d[:].rearrange("(h p) c -> p h c", p=128)
                    for og in range(4):
                        nc.gpsimd.dma_start(out=wo_sb[:, :, og * 512:(og + 1) * 512],
                                            in_=wo_r[:, :, og * 512:(og + 1) * 512])

                    # ---- Phase 2: attention ----
                    with tc.tile_pool(name="ex", bufs=6) as ex, \
                         tc.tile_pool(name="sm", bufs=4) as sm, \
                         tc.tile_pool(name="p2s", bufs=4, space="PSUM") as p2s, \
                         tc.tile_pool(name="p2y", bufs=2, space="PSUM") as p2y, \
                         tc.tile_pool(name="p2n", bufs=2, space="PSUM") as p2n:

                        for w in range(NWIN):
                            njt = 4 * w + 4
                            for hq in range(HLOC):
                                kvh = hq // 4
                                ps_y = p2y.tile([128, 512], f32, tag="y")
                                # sums broadcast to all 128 partitions by the
                                # all-ones [128,128] stationary
                                ps_s = p2n.tile([128, 512], f32, tag="s")
                                rhs_q = qt_sb[:, hq, w * 512:(w + 1) * 512]
                                sc_tiles = {}

                                def emit_sc(j):
                                    ps_sc = p2s.tile([128, 512], f32, tag="sc")
                                    nc.tensor.matmul(
                                        ps_sc[:], kt_sb[:, kvh, j, :], rhs_q)
                                    if j >= 4 * w:
                                        nc.vector.tensor_add(ps_sc[:], ps_sc[:],
                                                             masks[:, j - 4 * w, :])
                                    sc_tiles[j] = ps_sc

                                emit_sc(0)
                                for j in range(njt):
                                    if j + 1 < njt:
                                        emit_sc(j + 1)
                                    ps_sc = sc_tiles.pop(j)
                                    et = ex.tile([128, 512], bf16, tag="et")
                                    nc.scalar.activation(et[:], ps_sc[:],
                                                         mybir.ActivationFunctionType.Exp,
                                                         scale=SCALE)
                                    st, sp = j == 0, j == njt - 1
                                    nc.tensor.matmul(
                                        ps_y[:],
                                        v_sb[:, j, kvh * 128:(kvh + 1) * 128],
                                        et[:], start=st, stop=sp,
                                        skip_group_check=True)
                                    nc.tensor.matmul(
                                        ps_s[:], ones[:], et[:],
                                        start=st, stop=sp, skip_group_check=True)

                                rec = sm.tile([128, 512], f32, tag="rec")
                                nc.vector.reciprocal_approx_fast(rec[:], ps_s[:])
                                nc.vector.tensor_mul(
                                    ytn[:, hq, w * 512:(w + 1) * 512], ps_y[:], rec[:])

                    # ---- Phase 3: output projection ----
                    with tc.tile_pool(name="ob", bufs=4) as ob, \
                         tc.tile_pool(name="p3", bufs=4, space="PSUM") as p3:
                        for tcid in range(NTC):
                            for og in range(4):
                                ps_o = p3.tile([128, 512], f32, tag="o")
                                for h in range(HLOC):
                                    nc.tensor.matmul(
                                        ps_o[:],
                                        ytn[:, h, tcid * 128:(tcid + 1) * 128],
                                        wo_sb[:, h, og * 512:(og + 1) * 512],
                                        start=(h == 0), stop=(h == HLOC - 1))
                                ot = ob.tile([128, 512], f32, tag="ot")
                                nc.scalar.activation(ot[:], ps_o[:],
                                                     mybir.ActivationFunctionType.Copy)
                                q_eng = nc.gpsimd if og % 2 == 0 else nc.sync
                                q_eng.dma_start(
                                    out=out_d[tcid * 128:(tcid + 1) * 128, og * 512:(og + 1) * 512],
                                    in_=ot[:])

    nc.compile()
    return nc


_PROGRAM = None


def _get_program():
    global _PROGRAM
    if _PROGRAM is None:
        _PROGRAM = build_program()
    return _PROGRAM


def make_in_maps(x, W_qkv, W_out):
    bf = _np_bf16()
    x = np.asarray(x, dtype=np.float32)
    W_qkv = np.asarray(W_qkv, dtype=np.float32)
    W_out = np.asarray(W_out, dtype=np.float32)
    in_maps = []
    for c in range(8):
        b, t = c // 2, c % 2
        xt = np.ascontiguousarray(x[b].T).astype(bf)
        wq = np.ascontiguousarray(W_qkv[:, t * 1024:(t + 1) * 1024]).astype(bf)
        wk = W_qkv[:, D + t * 256: D + (t + 1) * 256]
        wv = W_qkv[:, D + 512 + t * 256: D + 512 + (t + 1) * 256]
        wkv = np.ascontiguousarray(np.concatenate([wk, wv], axis=1)).astype(bf)
        wo = np.ascontiguousarray(W_out[t * 1024:(t + 1) * 1024, :]).astype(bf)
        in_maps.append({"xt": xt, "wq": wq, "wkv": wkv, "wo": wo})
    return in_maps


def kernel(x, W_qkv, W_out):
    from concourse.bass_utils import run_bass_kernel_spmd
    nc = _get_program()
    in_maps = make_in_maps(x, W_qkv, W_out)
    res = run_bass_kernel_spmd(nc, in_maps, list(range(8)), trace=False)
    out = np.empty((B, S, D), dtype=np.float32)
    for b in range(B):
        out[b] = res.results[2 * b]["out"] + res.results[2 * b + 1]["out"]
    return out
